# revision 1
# baseline (speedup 1.0000x reference)
"""Trainium2 Bass kernel for nn_Meta_67078799229377 (relation-network meta-learner).

Sharding: 8 cores = 4 batch elements x 2 halves of the relation-j axis.
Each core runs the full backbone for its batch element's 6 images, then the
relation network for its 18 (i, j) pairs, fully fused on-chip (the
[s,s,m,m,128] tensor never exists in HBM). Host code only reshapes/shards
inputs and combines 144 scores + 24 per-sample CE terms into the 3 scalar
losses.
"""
import os
import numpy as np
import ml_dtypes

import concourse.bass as bass
import concourse.mybir as mybir
import concourse.tile as tile
from concourse import bacc
from concourse.bass_utils import run_bass_kernel_spmd

F32 = mybir.dt.float32
F32R = mybir.dt.float32r
BF16 = mybir.dt.bfloat16
AF = mybir.ActivationFunctionType
OP = mybir.AluOpType

B, S, D = 4, 6, 8
M = D * D            # 64 spatial positions
C2 = 66              # 64 channels + 2 coord channels
H1 = 128             # g-MLP hidden
CO = 64              # g-MLP out
NCls = 64
N_CORES = 8

# Fraction of hdd-gen ops routed to the scalar engine (rest on vector engine).
ACT_HDD_EVERY = 5    # every 5th q goes to ACT


def _build_nc():
    nc = bacc.Bacc("TRN2", target_bir_lowering=False, debug=False,
                   num_devices=N_CORES)

    din = {}
    def dram_in(name, shape, dtype=F32):
        din[name] = nc.dram_tensor(name, list(shape), dtype, kind="ExternalInput")
        return din[name]

    x_patches = dram_in("patches", [27, S, 1024], BF16)
    x_w1 = dram_in("w1", [27, 32], BF16)
    x_w2 = dram_in("w2", [32, 9 * 48], BF16)
    x_w3 = dram_in("w3", [48, 9 * 64], BF16)
    x_bc1 = dram_in("bc1", [32, 1])
    x_bc2 = dram_in("bc2", [48, 1])
    x_bc3 = dram_in("bc3", [64, 1])
    x_coords = dram_in("coords", [2, S * M], BF16)
    x_wle = dram_in("wle", [65, NCls])
    x_onehot = dram_in("onehot", [S, NCls])
    x_w1a = dram_in("w1a", [C2, H1], BF16)
    x_w1b = dram_in("w1b", [C2, H1], BF16)
    x_bg1 = dram_in("bg1", [H1, 1])
    x_wg2 = dram_in("wg2", [H1, CO], BF16)
    x_bg2 = dram_in("bg2_2", [2 * CO, 1])
    x_wf1e = dram_in("wf1e", [65, 16])
    x_wf2e = dram_in("wf2e", [17, 1])

    out_scores = nc.dram_tensor("scores", [18, 1], F32, kind="ExternalOutput")
    out_cls = nc.dram_tensor("clsv", [S, 1], F32, kind="ExternalOutput")

    with tile.TileContext(nc) as tc:
        with (
            tc.tile_pool(name="const", bufs=1) as cpool,
            tc.tile_pool(name="work", bufs=1) as wpool,
            tc.tile_pool(name="patch", bufs=1) as ppool,
            tc.tile_pool(name="hdd", bufs=2) as hpool,
            tc.tile_pool(name="gscr", bufs=2) as spool,
            tc.tile_pool(name="pconv", bufs=2, space="PSUM") as pc_pool,
            tc.tile_pool(name="pbig", bufs=2, space="PSUM") as pb_pool,
            tc.tile_pool(name="psmall", bufs=2, space="PSUM") as ps_pool,
        ):
            # ---- constants to SBUF ----
            def c_tile(src, shape, dtype=F32):
                t = cpool.tile(list(shape), dtype, tag=src.name)
                nc.sync.dma_start(out=t[:], in_=src[:])
                return t

            w1_sb = c_tile(x_w1, [27, 32], BF16)
            w2_sb = c_tile(x_w2, [32, 9 * 48], BF16)
            w3_sb = c_tile(x_w3, [48, 9 * 64], BF16)
            bc1_sb = c_tile(x_bc1, [32, 1])
            bc2_sb = c_tile(x_bc2, [48, 1])
            bc3_sb = c_tile(x_bc3, [64, 1])
            wle_sb = c_tile(x_wle, [65, NCls])
            onehot_sb = c_tile(x_onehot, [S, NCls])
            w1a_sb = c_tile(x_w1a, [C2, H1], BF16)
            w1b_sb = c_tile(x_w1b, [C2, H1], BF16)
            bg1_sb = c_tile(x_bg1, [H1, 1])
            wg2_sb = c_tile(x_wg2, [H1, CO], BF16)
            bg2_sb = c_tile(x_bg2, [2 * CO, 1])
            wf1e_sb = c_tile(x_wf1e, [65, 16])
            wf2e_sb = c_tile(x_wf2e, [17, 1])

            patches_sb = ppool.tile([27, S, 1024], BF16)
            nc.sync.dma_start(out=patches_sb[:], in_=x_patches[:])

            featc = wpool.tile([C2, S * M], BF16)
            nc.sync.dma_start(out=featc[64:66, :], in_=x_coords[:])

            def r32(ap):
                return ap

            _stages = ["c1", "c2", "c3", "cls", "uv", "rel", "full"]
            _stop = os.environ.get("KSTOP", "full")
            def _do(stage):
                return _stages.index(stage) <= _stages.index(_stop)


            # ---- conv1: [27]->[32], 64x64 -> 32x32 (stride 2, im2col'd) ----
            c1sb = wpool.tile([32, S, 33, 33], BF16)
            for img in range(S):
                # zero the padding strip (row 32 and col 32)
                nc.gpsimd.memset(c1sb[:, img, 32, :], 0.0)
                nc.gpsimd.memset(c1sb[:, img, 0:32, 32], 0.0)
            for img in range(S):
                for h in range(2):
                    ps1 = pc_pool.tile([32, 16, 32], F32, tag="psc")
                    nc.tensor.matmul(
                        ps1[:].rearrange("p a b -> p (a b)"),
                        r32(w1_sb[:]),
                        r32(patches_sb[:, img, h * 512:(h + 1) * 512]),
                        start=True, stop=True)
                    # relu(x + bc1) -> padded layout; alternate engines
                    out_ap = c1sb[:, img, h * 16:(h + 1) * 16, 0:32]
                    if img % 2 == 0:
                        nc.scalar.activation(out_ap, ps1[:], AF.Relu, bias=bc1_sb[:])
                    else:
                        nc.vector.tensor_scalar(out_ap, ps1[:], bc1_sb[:], 0.0,
                                                op0=OP.add, op1=OP.max)

            if _do("c2"):
                # ---- conv2: [32]->[48], 32x32 -> 16x16 ----
                c2sb = wpool.tile([48, S, 17, 17], BF16)
                for img in range(S):
                    nc.gpsimd.memset(c2sb[:, img, 16, :], 0.0)
                    nc.gpsimd.memset(c2sb[:, img, 0:16, 16], 0.0)
                for ip in range(3):      # image pairs
                    ps2 = pc_pool.tile([48, 2, 16, 16], F32, tag="psc")
                    for k, (dy, dx) in enumerate((dy, dx) for dy in range(3) for dx in range(3)):
                        nc.tensor.matmul(
                            ps2[:],
                            r32(w2_sb[:, k * 48:(k + 1) * 48]),
                            r32(c1sb[:, 2 * ip:2 * ip + 2, dy:dy + 31:2, dx:dx + 31:2]),
                            start=(k == 0), stop=(k == 8))
                    out_ap = c2sb[:, 2 * ip:2 * ip + 2, 0:16, 0:16]
                    if ip % 2 == 0:
                        nc.scalar.activation(out_ap, ps2[:], AF.Relu, bias=bc2_sb[:])
                    else:
                        nc.vector.tensor_scalar(out_ap, ps2[:], bc2_sb[:], 0.0,
                                                op0=OP.add, op1=OP.max)

            if _do("c3"):
                # ---- conv3: [48]->[64], 16x16 -> 8x8 ----
                ps3 = ps_pool.tile([64, S, D, D], F32, tag="sm")
                for k, (dy, dx) in enumerate((dy, dx) for dy in range(3) for dx in range(3)):
                    nc.tensor.matmul(
                        ps3[:],
                        r32(w3_sb[:, k * 64:(k + 1) * 64]),
                        r32(c2sb[:, :, dy:dy + 15:2, dx:dx + 15:2]),
                        start=(k == 0), stop=(k == 8))
                nc.scalar.activation(featc[0:64, :].rearrange("p (i m) -> p i m", m=M),
                                     ps3[:].rearrange("p i a b -> p i (a b)"),
                                     AF.Relu, bias=bc3_sb[:])

            if _do("cls"):
                # ---- cls head ----
                fme = wpool.tile([65, S], F32)
                nc.gpsimd.memset(fme[:], 1.0)
                nc.vector.tensor_reduce(
                    fme[0:64, :], featc[0:64, :].rearrange("p (i m) -> p i m", m=M),
                    axis=mybir.AxisListType.X, op=OP.add)
                psl = ps_pool.tile([S, NCls], F32, tag="sm")
                nc.tensor.matmul(psl[:], r32(fme[:]), r32(wle_sb[:]), start=True, stop=True)
                mx = wpool.tile([S, 1], F32)
                nc.vector.tensor_reduce(mx[:], psl[:], axis=mybir.AxisListType.X, op=OP.max)
                shifted = wpool.tile([S, NCls], F32)
                nc.vector.tensor_scalar(shifted[:], psl[:], mx[:], None, op0=OP.subtract)
                escr = wpool.tile([S, NCls], F32)
                se = wpool.tile([S, 1], F32)
                nc.scalar.activation(escr[:], shifted[:], AF.Exp, accum_out=se[:])
                lse = wpool.tile([S, 1], F32)
                nc.scalar.activation(lse[:], se[:], AF.Ln)
                selscr = wpool.tile([S, NCls], F32)
                sel = wpool.tile([S, 1], F32)
                nc.vector.tensor_tensor(selscr[:], shifted[:], onehot_sb[:], op=OP.mult)
                nc.vector.tensor_reduce(sel[:], selscr[:], axis=mybir.AxisListType.X, op=OP.add)
                clsv = wpool.tile([S, 1], F32)
                nc.vector.tensor_tensor(clsv[:], lse[:], sel[:], op=OP.subtract)
                nc.sync.dma_start(out=out_cls[:], in_=clsv[:])

            if _do("uv"):
                # ---- u / v ----
                psu = ps_pool.tile([H1, S * M], F32, tag="sm")
                psv = ps_pool.tile([H1, S * M], F32, tag="sm")
                nc.tensor.matmul(psu[:], r32(w1a_sb[:]), r32(featc[:]), start=True, stop=True)
                nc.tensor.matmul(psv[:], r32(w1b_sb[:]), r32(featc[:]), start=True, stop=True)
                u_f32 = wpool.tile([H1, S * M], F32)
                v_bf = wpool.tile([H1, S * M], BF16)
                v_f32 = wpool.tile([H1, S * M], F32)
                nc.scalar.activation(u_f32[:], psu[:], AF.Copy)
                nc.vector.tensor_scalar(v_bf[:], psv[:], bg1_sb[:], None, op0=OP.add)
                nc.vector.tensor_scalar(v_f32[:], psv[:], bg1_sb[:], None, op0=OP.add)

            if _do("rel"):
                # ---- relation stage ----
                xf_cols = wpool.tile([2 * CO, 36], F32)
                nc.gpsimd.memset(xf_cols[:], 0.0)
                max_units = int(os.environ.get("KUNITS", "6"))
                unit_no = 0
                for jl in range(3):
                    for qh in range(2):
                        unit_no += 1
                        if unit_no > max_units:
                            continue
                        hdd = hpool.tile([H1, 32, S * M], BF16, tag="hdd")
                        for ql in range(32):
                            q = qh * 32 + ql
                            ucol = u_f32[:, jl * M + q: jl * M + q + 1]
                            if ql % ACT_HDD_EVERY == ACT_HDD_EVERY - 1:
                                nc.scalar.activation(hdd[:, ql, :], v_f32[:],
                                                     AF.Relu, bias=ucol)
                            else:
                                nc.vector.tensor_scalar(hdd[:, ql, :], v_bf[:],
                                                        ucol, 0.0,
                                                        op0=OP.add, op1=OP.max)
                        for duo in range(3):
                            iA, iB = 2 * duo, 2 * duo + 1
                            for gh in range(2):
                                ps = pb_pool.tile([2 * CO, 1024], F32, tag="gps")
                                for q2 in range(2):
                                    qg = gh * 2 + q2
                                    nc.tensor.matmul(
                                        ps[0:CO, q2 * 512:(q2 + 1) * 512],
                                        wg2_sb[:],
                                        hdd[:, qg * 8:(qg + 1) * 8, iA * M:(iA + 1) * M],
                                        start=True, stop=True)
                                    nc.tensor.matmul(
                                        ps[CO:2 * CO, q2 * 512:(q2 + 1) * 512],
                                        wg2_sb[:],
                                        hdd[:, qg * 8:(qg + 1) * 8, iB * M:(iB + 1) * M],
                                        start=True, stop=True,
                                        tile_position=(0, 64))
                                ucol_i = (((jl * 2 + qh) * 3 + duo) * 2) + gh
                                gscr = spool.tile([2 * CO, 1024], BF16, tag="gscr")
                                nc.scalar.activation(gscr[:], ps[:], AF.Relu,
                                                     bias=bg2_sb[:],
                                                     accum_out=xf_cols[:, ucol_i:ucol_i + 1])

            if _do("rel"):
                # ---- score head ----
                # sum the two gh-halves, then the two qh-halves
                xf18 = wpool.tile([2 * CO, 18], F32)
                nc.vector.tensor_tensor(
                    xf18[:],
                    xf_cols[:].rearrange("p (a g) -> p a g", g=2)[:, :, 0],
                    xf_cols[:].rearrange("p (a g) -> p a g", g=2)[:, :, 1],
                    op=OP.add)
                # xf_pair[:, jl*3+d] = xf18[:, jl*6+d] + xf18[:, jl*6+3+d]
                xf_pair = wpool.tile([2 * CO, 3, 3], F32)
                nc.vector.tensor_tensor(
                    xf_pair[:],
                    xf18[:].rearrange("p (a b) -> p a b", a=6)[:, 0:6:2, :],
                    xf18[:].rearrange("p (a b) -> p a b", a=6)[:, 1:6:2, :],
                    op=OP.add)
                xf_ext = wpool.tile([65, 18], F32)
                nc.gpsimd.memset(xf_ext[:], 1.0)
                # even local-pair columns <- partitions 0:64 (i = 2d)
                nc.vector.tensor_copy(
                    xf_ext[0:64, :].rearrange("p (a b) -> p a b", a=3)[:, :, 0:6:2],
                    xf_pair[0:64, :, :])
                # odd local-pair columns <- partitions 64:128 (i = 2d+1), needs DMA
                nc.sync.dma_start(
                    out=xf_ext[0:64, :].rearrange("p (a b) -> p a b", a=3)[:, :, 1:6:2],
                    in_=xf_pair[64:128, :, :])
                psh1 = ps_pool.tile([16, 18], F32, tag="sm")
                nc.tensor.matmul(psh1[:], r32(wf1e_sb[:]), r32(xf_ext[:]),
                                 start=True, stop=True)
                h1e = wpool.tile([17, 18], F32)
                nc.gpsimd.memset(h1e[:], 1.0)
                nc.scalar.activation(h1e[0:16, :], psh1[:], AF.Relu)
                psh2 = ps_pool.tile([18, 1], F32, tag="sm")
                nc.tensor.matmul(psh2[:], r32(h1e[:]), r32(wf2e_sb[:]),
                                 start=True, stop=True)
                en = wpool.tile([18, 1], F32)
                nc.scalar.activation(en[:], psh2[:], AF.Exp, scale=-1.0)
                ep1 = wpool.tile([18, 1], F32)
                nc.vector.tensor_scalar(ep1[:], en[:], 1.0, None, op0=OP.add)
                sc = wpool.tile([18, 1], F32)
                nc.vector.reciprocal(sc[:], ep1[:])
                nc.sync.dma_start(out=out_scores[:], in_=sc[:])

            if not _do("cls"):
                d2 = wpool.tile([S, 1], F32, tag="dummy2")
                nc.gpsimd.memset(d2[:], 0.0)
                nc.sync.dma_start(out=out_cls[:], in_=d2[:])
            if not _do("rel"):
                d1 = wpool.tile([18, 1], F32, tag="dummy1")
                nc.gpsimd.memset(d1[:], 0.0)
                nc.sync.dma_start(out=out_scores[:], in_=d1[:])
    nc.compile()
    return nc


_NC_CACHE = None


def _get_nc():
    global _NC_CACHE
    if _NC_CACHE is None:
        _NC_CACHE = _build_nc()
    return _NC_CACHE


def _host_prep(inputs):
    ins = {k: np.asarray(v) for k, v in inputs.items()}
    x = np.concatenate([ins['support_x'], ins['query_x']], axis=1)
    lab = np.concatenate([ins['support_y'], ins['query_y']], axis=1)

    xpad = np.pad(x.astype(np.float32), ((0, 0), (0, 0), (0, 0), (0, 1), (0, 1)))
    win = np.lib.stride_tricks.sliding_window_view(xpad, (3, 3), axis=(3, 4))
    win = win[:, :, :, ::2, ::2]
    patches = win.transpose(0, 2, 5, 6, 1, 3, 4).reshape(B, 27, S, 1024)
    patches = np.ascontiguousarray(patches, np.float32)

    f32 = np.float32
    bf16 = ml_dtypes.bfloat16
    w1 = np.ascontiguousarray(ins['k1'].reshape(32, 27).T, f32).astype(bf16)
    w2 = np.ascontiguousarray(ins['k2'].transpose(1, 2, 3, 0).reshape(32, 9 * 48), f32).astype(bf16)
    w3 = np.ascontiguousarray(ins['k3'].transpose(1, 2, 3, 0).reshape(48, 9 * 64), f32).astype(bf16)

    ii = np.arange(D, dtype=f32) / D
    coord = np.stack([np.broadcast_to(ii[:, None], (D, D)),
                      np.broadcast_to(ii[None, :], (D, D))]).reshape(2, M)
    coords = np.ascontiguousarray(np.tile(coord, (1, S)), f32).astype(bf16)

    onehots = np.zeros((B, S, NCls), f32)
    for b in range(B):
        onehots[b, np.arange(S), lab[b]] = 1.0

    Wg1 = ins['Wg1'].astype(f32)
    common = dict(
        w1=w1, w2=w2, w3=w3,
        bc1=np.ascontiguousarray(ins['bc1'].reshape(32, 1), f32),
        bc2=np.ascontiguousarray(ins['bc2'].reshape(48, 1), f32),
        bc3=np.ascontiguousarray(ins['bc3'].reshape(64, 1), f32),
        coords=coords,
        wle=np.ascontiguousarray(
            np.vstack([ins['Wlog'].astype(f32) / M, ins['blog'][None, :].astype(f32)])),
        w1a=np.ascontiguousarray(Wg1[:C2]).astype(bf16),
        w1b=np.ascontiguousarray(Wg1[C2:]).astype(bf16),
        bg1=np.ascontiguousarray(ins['bg1'].reshape(H1, 1), f32),
        wg2=np.ascontiguousarray(ins['Wg2'], f32).astype(ml_dtypes.bfloat16),
        bg2_2=np.ascontiguousarray(np.tile(ins['bg2'].astype(f32), 2).reshape(2 * CO, 1)),
        wf1e=np.ascontiguousarray(
            np.vstack([ins['Wf1'].astype(f32), ins['bf1'][None, :].astype(f32)])),
        wf2e=np.ascontiguousarray(
            np.vstack([ins['Wf2'].astype(f32), ins['bf2'].reshape(1, 1).astype(f32)])),
    )
    in_maps = []
    for core in range(N_CORES):
        b, half = core // 2, core % 2
        # odd cores see images in rotated order so the program's local
        # j in {0,1,2} maps to global j in {3,4,5}
        perm = (0, 1, 2, 3, 4, 5) if half == 0 else (3, 4, 5, 0, 1, 2)
        m = dict(common)
        m['patches'] = np.ascontiguousarray(patches[b][:, perm, :]).astype(ml_dtypes.bfloat16)
        m['onehot'] = np.ascontiguousarray(onehots[b][list(perm)])
        in_maps.append(m)
    return in_maps, lab


def _host_post(results, lab):
    P = np.zeros((B, S, S), np.float32)
    cls_terms = np.zeros((B, S), np.float32)
    for core in range(N_CORES):
        b, half = core // 2, core % 2
        perm = (0, 1, 2, 3, 4, 5) if half == 0 else (3, 4, 5, 0, 1, 2)
        sc = results[core]["scores"].reshape(18)
        for jl in range(3):
            for i in range(S):
                P[b, perm[i], perm[jl]] = sc[jl * 6 + i]
        if half == 0:
            cls_terms[b] = results[core]["clsv"].reshape(S)
    cls_loss = np.float32(cls_terms.mean())
    y = (lab[:, :, None] == lab[:, None, :]).astype(np.float32)
    Pt = P.transpose(0, 2, 1)
    sym, anti = np.float32(0.5) * (P + Pt), np.float32(0.5) * (P - Pt)
    sym_n = np.sqrt((sym ** 2).sum(axis=(1, 2)))
    anti_n = np.sqrt((anti ** 2).sum(axis=(1, 2)))
    sym_loss = np.float32(((sym_n - anti_n) / (sym_n + anti_n)).mean())
    euc_loss = np.float32(((P - y) ** 2).mean())
    rn_loss = np.float32(euc_loss - np.float32(0.1) * sym_loss)
    return np.float32(cls_loss), np.float32(rn_loss), np.float32(sym_loss)


def run_spmd(inputs, trace=False, **kwargs):
    nc = _get_nc()
    in_maps, lab = _host_prep(inputs)
    res = run_bass_kernel_spmd(nc, in_maps, list(range(N_CORES)),
                               trace=trace, **kwargs)
    return _host_post(res.results, lab), res


def kernel(**inputs):
    out, _ = run_spmd(inputs)
    return out



# revision 5
# speedup vs baseline: 1.5560x; 1.5560x over previous
"""Trainium2 Bass kernel for nn_Meta_67078799229377 (relation-network meta-learner).

Sharding: 8 cores = 4 batch elements x 2 halves of the relation-j axis.
v2: packed input DMAs + PE warmup, row/col-tiled convs, balanced DVE/ACT
relation stage (hdd on DVE, fused relu+sum z-blocks on ACT), matmul tail.
"""
import os
import numpy as np
import ml_dtypes

import concourse.bass as bass
import concourse.mybir as mybir
import concourse.tile as tile
from concourse import bacc
from concourse.bass_utils import run_bass_kernel_spmd

F32 = mybir.dt.float32
BF16 = mybir.dt.bfloat16
AF = mybir.ActivationFunctionType
OP = mybir.AluOpType

B, S, D = 4, 6, 8
M = D * D
C2 = 66
H1 = 128
CO = 64
NCls = 64
N_CORES = 8

# wb (bf16 pack) column offsets
W1C, W2C, W3C = 0, 32, 464
W1A, W1B, WG2, WF1, WF2 = 1040, 1168, 1296, 1360, 1376
CRD = 1377
NB = 1761
# wf (f32 pack) column offsets
BC1, BC2, BC3, BG1, BG2, BF1, NBF2, WLE, OH = 0, 1, 2, 3, 4, 5, 6, 7, 71
NF = 135

WARMUP_MMS = int(os.environ.get("KWARM", "14"))
ACT_QS = (10, 21)   # per-unit hdd q-ops routed to the scalar engine


def _build_nc():
    nc = bacc.Bacc("TRN2", target_bir_lowering=False, debug=False,
                   num_devices=N_CORES)

    x_wb = nc.dram_tensor("wb", [128, NB], BF16, kind="ExternalInput")
    x_wf = nc.dram_tensor("wf", [128, NF], F32, kind="ExternalInput")
    x_pt = nc.dram_tensor("pt", [128, 2, 2, 512], BF16, kind="ExternalInput")
    out_scores = nc.dram_tensor("scores", [18, 1], F32, kind="ExternalOutput")
    out_cls = nc.dram_tensor("clsv", [S, 1], F32, kind="ExternalOutput")

    with tile.TileContext(nc) as tc:
        with (
            tc.tile_pool(name="const", bufs=1) as cpool,
            tc.tile_pool(name="work", bufs=1) as wpool,
            tc.tile_pool(name="hdd", bufs=2) as hpool,
            tc.tile_pool(name="gscr", bufs=2) as spool,
            tc.tile_pool(name="pz", bufs=2, space="PSUM") as pz,
        ):
            # ---- PE warmup + ACT table preloads (no input deps) ----
            warm = wpool.tile([128, 576], BF16)
            nc.vector.memset(warm[:], 0.001)
            dum = wpool.tile([2, 4], F32)
            nc.scalar.activation(dum[:, 0:2], warm[0:2, 0:2], AF.Exp)
            nc.scalar.activation(dum[:, 2:4], dum[:, 0:2], AF.Ln)
            nc.scalar.activation(dum[:, 0:2], dum[:, 2:4], AF.Relu)
            pswarm = pz.tile([128, 2048], F32, tag="z")
            for r in range(WARMUP_MMS):
                nc.tensor.matmul(pswarm[0:64, 0:512], warm[:, 0:64],
                                 warm[:, 64:576], start=True, stop=True)

            # ---- inputs ----
            wb = cpool.tile([128, NB], BF16)
            nc.sync.dma_start(out=wb[:], in_=x_wb[:])
            pt = cpool.tile([128, 2, 2, 512], BF16)
            nc.gpsimd.dma_start(out=pt[:], in_=x_pt[:])
            wf = cpool.tile([128, NF], F32)
            nc.scalar.dma_start(out=wf[:], in_=x_wf[:])

            # ---- conv buffers + padding ----
            c1a = wpool.tile([128, 33, 33], BF16)   # imgs 0-3, ch at part 32k
            c1b = wpool.tile([64, 33, 33], BF16)    # imgs 4-5
            nc.gpsimd.memset(c1a[:, 32, :], 0.0)
            nc.gpsimd.memset(c1a[:, 0:32, 32], 0.0)
            nc.gpsimd.memset(c1b[:, 32, :], 0.0)
            nc.gpsimd.memset(c1b[:, 0:32, 32], 0.0)
            c2f = wpool.tile([128, 4, 17, 17], BF16)  # imgs0-3 @0:48, 4-5 @64:112
            nc.gpsimd.memset(c2f[:, :, 16, :], 0.0)
            nc.gpsimd.memset(c2f[:, :, 0:16, 16], 0.0)

            # ---- conv1: 27->32ch, 64x64 -> 32x32, diag-tiled 4 imgs ----
            c1ps = pz.tile([128, 2, 16, 32], F32, tag="z")
            for h in range(2):
                for k in range(4):
                    nc.tensor.matmul(
                        c1ps[32 * k:32 * k + 32, h, :, :],
                        wb[32 * k:32 * k + 27, W1C:W1C + 32],
                        pt[32 * k:32 * k + 27, h, 0, :],
                        start=True, stop=True, tile_position=(32 * k, 32 * k))
            nc.scalar.activation(
                c1a[:, 0:32, 0:32].rearrange("p (h y) x -> p h y x", h=2),
                c1ps[:], AF.Relu, bias=wf[:, BC1:BC1 + 1])
            c1psb = pz.tile([64, 2, 16, 32], F32, tag="z")
            for h in range(2):
                for k in range(2):
                    nc.tensor.matmul(
                        c1psb[32 * k:32 * k + 32, h, :, :],
                        wb[32 * k:32 * k + 27, W1C:W1C + 32],
                        pt[32 * k:32 * k + 27, h, 1, :],
                        start=True, stop=True)
            nc.vector.tensor_scalar(
                c1b[:, 0:32, 0:32].rearrange("p (h y) x -> p h y x", h=2),
                c1psb[:], wf[0:64, BC1:BC1 + 1], 0.0, op0=OP.add, op1=OP.max)

            # ---- conv2: 32->48ch, 32x32 -> 16x16, 4-way row-tiled ----
            c2ps = pz.tile([48, 4, 512], F32, tag="z")
            taps = [(dy, dx) for dy in range(3) for dx in range(3)]
            for t, (dy, dx) in enumerate(taps):
                for k in range(4):
                    nc.tensor.matmul(
                        c2ps[:, k, 0:256].rearrange("p (y x) -> p y x", y=16),
                        wb[32 * k:32 * k + 32, W2C + 48 * t:W2C + 48 * (t + 1)],
                        c1a[32 * k:32 * k + 32, dy:dy + 31:2, dx:dx + 31:2],
                        start=(t == 0), stop=(t == 8), skip_group_check=True,
                        tile_position=(32 * k, 0))
            nc.scalar.activation(
                c2f[0:48, :, 0:16, 0:16],
                c2ps[:, :, 0:256].rearrange("p k (y x) -> p k y x", y=16),
                AF.Relu, bias=wf[0:48, BC2:BC2 + 1])
            c2psb = pz.tile([128, 2, 512], F32, tag="z")
            for t, (dy, dx) in enumerate(taps):
                for k in range(2):
                    nc.tensor.matmul(
                        c2psb[64:112, k, 0:256].rearrange("p (y x) -> p y x", y=16),
                        wb[32 * k:32 * k + 32, W2C + 48 * t:W2C + 48 * (t + 1)],
                        c1b[32 * k:32 * k + 32, dy:dy + 31:2, dx:dx + 31:2],
                        start=(t == 0), stop=(t == 8), skip_group_check=True)
            nc.vector.tensor_scalar(
                c2f[64:112, 0:2, 0:16, 0:16],
                c2psb[64:112, :, 0:256].rearrange("p k (y x) -> p k y x", y=16),
                wf[64:112, BC2:BC2 + 1], 0.0, op0=OP.add, op1=OP.max)

            # ---- conv3: 48->64ch, 16x16 -> 8x8, 2 row-tile streams ----
            c3psa = pz.tile([64, 4, 64], F32, tag="z")
            c3psb = pz.tile([64, 2, 64], F32, tag="z")
            for t, (dy, dx) in enumerate(taps):
                nc.tensor.matmul(
                    c3psa[:].rearrange("p k (y x) -> p k y x", y=8),
                    wb[0:48, W3C + 64 * t:W3C + 64 * (t + 1)],
                    c2f[0:48, :, dy:dy + 15:2, dx:dx + 15:2],
                    start=(t == 0), stop=(t == 8), skip_group_check=True)
                nc.tensor.matmul(
                    c3psb[:].rearrange("p k (y x) -> p k y x", y=8),
                    wb[64:112, W3C + 64 * t:W3C + 64 * (t + 1)],
                    c2f[64:112, 0:2, dy:dy + 15:2, dx:dx + 15:2],
                    start=(t == 0), stop=(t == 8), skip_group_check=True)
            featc = wpool.tile([66, 6, M], BF16)
            nc.scalar.activation(featc[0:64, 0:4, :], c3psa[:], AF.Relu,
                                 bias=wf[0:64, BC3:BC3 + 1])
            nc.vector.tensor_scalar(featc[0:64, 4:6, :], c3psb[:],
                                    wf[0:64, BC3:BC3 + 1], 0.0,
                                    op0=OP.add, op1=OP.max)
            nc.vector.tensor_copy(featc[64:66, :, :],
                                  wb[64:66, CRD:CRD + 384]
                                  .rearrange("p (i m) -> p i m", m=M))

            # ---- u / v ----
            fc = featc[:].rearrange("p i m -> p (i m)")
            psu = pz.tile([128, 384], F32, tag="z")
            nc.tensor.matmul(psu[:], wb[0:66, W1A:W1A + 128], fc,
                             start=True, stop=True)
            psv = pz.tile([128, 384], F32, tag="z")
            nc.tensor.matmul(psv[:], wb[0:66, W1B:W1B + 128], fc,
                             start=True, stop=True)
            u_sb = wpool.tile([128, 384], F32)
            nc.scalar.activation(u_sb[:], psu[:], AF.Copy)
            v_bf = wpool.tile([128, 384], BF16)
            nc.vector.tensor_scalar(v_bf[:], psv[:], wf[:, BG1:BG1 + 1], None,
                                    op0=OP.add)

            # ---- cls head (runs whenever engines are free) ----
            fme = wpool.tile([65, S], F32)
            nc.gpsimd.memset(fme[64:65, :], 1.0)
            nc.vector.tensor_reduce(fme[0:64, :], featc[0:64, :, :],
                                    axis=mybir.AxisListType.X, op=OP.add)
            psl = pz.tile([S, NCls], F32, tag="z")
            nc.tensor.matmul(psl[:], fme[:], wf[0:65, WLE:WLE + 64],
                             start=True, stop=True)
            lsb = wpool.tile([S, NCls], F32)
            nc.vector.tensor_copy(lsb[:], psl[:])
            mx = wpool.tile([S, 1], F32)
            nc.vector.tensor_reduce(mx[:], lsb[:], axis=mybir.AxisListType.X,
                                    op=OP.max)
            shifted = wpool.tile([S, NCls], F32)
            nc.vector.tensor_scalar(shifted[:], lsb[:], mx[:], None,
                                    op0=OP.subtract)
            escr = wpool.tile([S, NCls], F32)
            se = wpool.tile([S, 1], F32)
            nc.scalar.activation(escr[:], shifted[:], AF.Exp, accum_out=se[:])
            lse = wpool.tile([S, 1], F32)
            nc.scalar.activation(lse[:], se[:], AF.Ln)
            selscr = wpool.tile([S, NCls], F32)
            sel = wpool.tile([S, 1], F32)
            nc.vector.tensor_tensor(selscr[:], shifted[:],
                                    wf[0:S, OH:OH + 64], op=OP.mult)
            nc.vector.tensor_reduce(sel[:], selscr[:],
                                    axis=mybir.AxisListType.X, op=OP.add)
            clsv = wpool.tile([S, 1], F32)
            nc.vector.tensor_tensor(clsv[:], lse[:], sel[:], op=OP.subtract)
            nc.sync.dma_start(out=out_cls[:], in_=clsv[:])

            # ---- relation units ----
            xf_cols = wpool.tile([128, 18], F32)
            for jl in range(3):
                for qh in range(2):
                    hdd = hpool.tile([128, 32, 384], BF16, tag="hdd")
                    for ql in range(32):
                        q = qh * 32 + ql
                        ucol = u_sb[:, jl * M + q:jl * M + q + 1]
                        if ql in ACT_QS:
                            nc.scalar.activation(hdd[:, ql, :], v_bf[:],
                                                 AF.Relu, bias=ucol)
                        else:
                            nc.vector.tensor_scalar(hdd[:, ql, :], v_bf[:],
                                                    ucol, 0.0,
                                                    op0=OP.add, op1=OP.max)
                    for duo in range(3):
                        iA, iB = 2 * duo, 2 * duo + 1
                        zps = pz.tile([128, 2048], F32, tag="z")
                        for ch in range(4):
                            q0 = ch * 8
                            nc.tensor.matmul(
                                zps[0:CO, 512 * ch:512 * (ch + 1)],
                                wb[:, WG2:WG2 + 64],
                                hdd[:, q0:q0 + 8, iA * M:(iA + 1) * M],
                                start=True, stop=True)
                            nc.tensor.matmul(
                                zps[CO:2 * CO, 512 * ch:512 * (ch + 1)],
                                wb[:, WG2:WG2 + 64],
                                hdd[:, q0:q0 + 8, iB * M:(iB + 1) * M],
                                start=True, stop=True)
                        gscr = spool.tile([128, 2048], BF16, tag="gscr")
                        col = (jl * 2 + qh) * 3 + duo
                        nc.scalar.activation(gscr[:], zps[:], AF.Relu,
                                             bias=wf[:, BG2:BG2 + 1],
                                             accum_out=xf_cols[:, col:col + 1])

            # ---- score head ----
            # xf_cols layout: col = (jl*2+qh)*3 + duo ; fold qh pairs
            xfq = wpool.tile([128, 9], F32)
            xv = xf_cols[:].rearrange("p (j h d) -> p j h d", j=3, h=2)
            nc.vector.tensor_tensor(
                xfq[:].rearrange("p (j d) -> p j d", j=3),
                xv[:, :, 0, :], xv[:, :, 1, :], op=OP.add)
            xfb = wpool.tile([128, 9], BF16)
            nc.vector.tensor_copy(xfb[:], xfq[:])
            psh1 = pz.tile([16, 18], F32, tag="z")
            nc.tensor.matmul(psh1[:, 0:9], wb[0:64, WF1:WF1 + 16],
                             xfb[0:64, :], start=True, stop=True)
            nc.tensor.matmul(psh1[:, 9:18], wb[64:128, WF1:WF1 + 16],
                             xfb[64:128, :], start=True, stop=True)
            h1s = wpool.tile([16, 18], BF16)
            nc.scalar.activation(h1s[:], psh1[:], AF.Relu,
                                 bias=wf[0:16, BF1:BF1 + 1])
            psh2 = pz.tile([18, 1], F32, tag="z")
            nc.tensor.matmul(psh2[:], h1s[:], wb[0:16, WF2:WF2 + 1],
                             start=True, stop=True)
            en = wpool.tile([18, 1], F32)
            nc.scalar.activation(en[:], psh2[:], AF.Exp, scale=-1.0,
                                 bias=wf[0:18, NBF2:NBF2 + 1])
            ep1 = wpool.tile([18, 1], F32)
            nc.vector.tensor_scalar(ep1[:], en[:], 1.0, None, op0=OP.add)
            sc = wpool.tile([18, 1], F32)
            nc.vector.reciprocal(sc[:], ep1[:])
            nc.sync.dma_start(out=out_scores[:], in_=sc[:])
    nc.compile()
    return nc


_NC_CACHE = None


def _get_nc():
    global _NC_CACHE
    if _NC_CACHE is None:
        _NC_CACHE = _build_nc()
    return _NC_CACHE


def _host_prep(inputs):
    f32 = np.float32
    bf16 = ml_dtypes.bfloat16
    ins = {k: np.asarray(v) for k, v in inputs.items()}
    x = np.concatenate([ins['support_x'], ins['query_x']], axis=1)
    lab = np.concatenate([ins['support_y'], ins['query_y']], axis=1)

    xpad = np.pad(x.astype(f32), ((0, 0), (0, 0), (0, 0), (0, 1), (0, 1)))
    win = np.lib.stride_tricks.sliding_window_view(xpad, (3, 3), axis=(3, 4))
    win = win[:, :, :, ::2, ::2]
    # [B, 27, S, 1024]
    patches = win.transpose(0, 2, 5, 6, 1, 3, 4).reshape(B, 27, S, 1024)
    patches = np.ascontiguousarray(patches, f32)

    w1 = np.ascontiguousarray(ins['k1'].reshape(32, 27).T, f32)
    w2 = np.ascontiguousarray(
        ins['k2'].transpose(1, 2, 3, 0).reshape(32, 9 * 48), f32)
    w3 = np.ascontiguousarray(
        ins['k3'].transpose(1, 2, 3, 0).reshape(48, 9 * 64), f32)
    Wg1 = ins['Wg1'].astype(f32)

    wb = np.zeros((128, NB), f32)
    for k in range(4):
        wb[32 * k:32 * k + 27, W1C:W1C + 32] = w1
        wb[32 * k:32 * k + 32, W2C:W2C + 432] = w2
    wb[0:48, W3C:W3C + 576] = w3
    wb[64:112, W3C:W3C + 576] = w3
    wb[0:66, W1A:W1A + 128] = Wg1[:C2]
    wb[0:66, W1B:W1B + 128] = Wg1[C2:]
    wb[0:128, WG2:WG2 + 64] = ins['Wg2'].astype(f32)
    wb[0:64, WF1:WF1 + 16] = ins['Wf1'].astype(f32)
    wb[64:128, WF1:WF1 + 16] = ins['Wf1'].astype(f32)
    wb[0:16, WF2:WF2 + 1] = ins['Wf2'].astype(f32)
    ii = np.arange(D, dtype=f32) / D
    coord = np.stack([np.broadcast_to(ii[:, None], (D, D)),
                      np.broadcast_to(ii[None, :], (D, D))]).reshape(2, M)
    wb[64:66, CRD:CRD + 384] = np.tile(coord, (1, S))
    wb = wb.astype(bf16)

    wfc = np.zeros((128, NF), f32)
    wfc[0:32, BC1] = ins['bc1'].astype(f32)
    wfc[0:48, BC2] = ins['bc2'].astype(f32)
    wfc[0:64, BC3] = ins['bc3'].astype(f32)
    wfc[64:128, BC3] = ins['bc3'].astype(f32)
    wfc[:, BG1] = ins['bg1'].astype(f32)
    wfc[0:64, BG2] = ins['bg2'].astype(f32)
    wfc[64:128, BG2] = ins['bg2'].astype(f32)
    wfc[0:16, BF1] = ins['bf1'].astype(f32)
    wfc[:, NBF2] = -float(ins['bf2'][0])
    wfc[0:64, WLE:WLE + 64] = ins['Wlog'].astype(f32) / M
    wfc[64, WLE:WLE + 64] = ins['blog'].astype(f32)

    onehots = np.zeros((B, S, NCls), f32)
    for b in range(B):
        onehots[b, np.arange(S), lab[b]] = 1.0

    in_maps = []
    for core in range(N_CORES):
        b, half = core // 2, core % 2
        perm = (0, 1, 2, 3, 4, 5) if half == 0 else (3, 4, 5, 0, 1, 2)
        p = patches[b][:, perm, :]          # [27, 6, 1024]
        pc = np.zeros((128, 2, 2, 512), f32)
        for k in range(4):
            pc[32 * k:32 * k + 27, :, 0, :] = \
                p[:, k, :].reshape(27, 2, 512)
        pc[0:27, :, 1, :] = p[:, 4, :].reshape(27, 2, 512)
        pc[32:59, :, 1, :] = p[:, 5, :].reshape(27, 2, 512)
        wfi = wfc.copy()
        wfi[0:S, OH:OH + 64] = onehots[b][list(perm)]
        in_maps.append(dict(wb=wb, wf=wfi, pt=pc.astype(bf16)))
    return in_maps, lab


def _host_post(results, lab):
    P = np.zeros((B, S, S), np.float32)
    cls_terms = np.zeros((B, S), np.float32)
    for core in range(N_CORES):
        b, half = core // 2, core % 2
        perm = (0, 1, 2, 3, 4, 5) if half == 0 else (3, 4, 5, 0, 1, 2)
        sc = results[core]["scores"].reshape(18)
        for r in range(18):
            ihalf, jl, duo = r // 9, (r % 9) // 3, r % 3
            P[b, perm[2 * duo + ihalf], perm[jl]] = sc[r]
        if half == 0:
            cls_terms[b] = results[core]["clsv"].reshape(S)
    cls_loss = np.float32(cls_terms.mean())
    y = (lab[:, :, None] == lab[:, None, :]).astype(np.float32)
    Pt = P.transpose(0, 2, 1)
    sym, anti = np.float32(0.5) * (P + Pt), np.float32(0.5) * (P - Pt)
    sym_n = np.sqrt((sym ** 2).sum(axis=(1, 2)))
    anti_n = np.sqrt((anti ** 2).sum(axis=(1, 2)))
    sym_loss = np.float32(((sym_n - anti_n) / (sym_n + anti_n)).mean())
    euc_loss = np.float32(((P - y) ** 2).mean())
    rn_loss = np.float32(euc_loss - np.float32(0.1) * sym_loss)
    return np.float32(cls_loss), np.float32(rn_loss), np.float32(sym_loss)


def run_spmd(inputs, trace=False, **kwargs):
    nc = _get_nc()
    in_maps, lab = _host_prep(inputs)
    res = run_bass_kernel_spmd(nc, in_maps, list(range(N_CORES)),
                               trace=trace, **kwargs)
    return _host_post(res.results, lab), res


def kernel(**inputs):
    out, _ = run_spmd(inputs)
    return out


# revision 15
# speedup vs baseline: 1.6731x; 1.0752x over previous
"""Trainium2 Bass kernel for nn_Meta_67078799229377 (relation-network meta-learner).

Sharding: 8 cores = 4 batch elements x 2 halves of the relation-j axis.
v2: packed input DMAs + PE warmup, row/col-tiled convs, balanced DVE/ACT
relation stage (hdd on DVE, fused relu+sum z-blocks on ACT), matmul tail.
"""
import os
import numpy as np
import ml_dtypes

import concourse.bass as bass
import concourse.mybir as mybir
import concourse.tile as tile
from concourse import bacc
from concourse.bass_utils import run_bass_kernel_spmd

F32 = mybir.dt.float32
BF16 = mybir.dt.bfloat16
AF = mybir.ActivationFunctionType
OP = mybir.AluOpType

B, S, D = 4, 6, 8
M = D * D
C2 = 66
H1 = 128
CO = 64
NCls = 64
N_CORES = 8

# wb (bf16 pack) column offsets
W1C, W2C, W3C = 0, 32, 464
W1A, W1B, WG2, WF1, WF2 = 1040, 1168, 1296, 1360, 1376
CRD = 1377
NB = 1761
# wf (f32 pack) column offsets
BC1, BC2, BC3, BG1, BG2, BF1, NBF2, WLE, OH = 0, 1, 2, 3, 4, 5, 6, 7, 71
NF = 135

WARMUP_MMS = int(os.environ.get("KWARM", "9"))
# per-unit hdd q-ops routed to the scalar engine: front-load unit 0 (ACT idle
# until the first z-block), tiny share mid-units, extra in the last unit
ACT_QS_BY_UNIT = {
    0: (9, 11, 17, 19, 25, 27, 29, 31),
    1: (21,), 2: (21,), 3: (21,), 4: (21,),
    5: (7, 15, 23),
}
# last unit: middle duo's relu+sum runs on DVE to parallelize the drain
DVE_Z = set()


def _build_nc():
    nc = bacc.Bacc("TRN2", target_bir_lowering=False, debug=False,
                   num_devices=N_CORES)

    x_wb = nc.dram_tensor("wb", [128, NB], BF16, kind="ExternalInput")
    x_wf = nc.dram_tensor("wf", [128, NF], F32, kind="ExternalInput")
    x_pt = nc.dram_tensor("pt", [128, 2, 2, 512], BF16, kind="ExternalInput")
    out_scores = nc.dram_tensor("scores", [18, 1], F32, kind="ExternalOutput")
    out_cls = nc.dram_tensor("logits", [S, NCls], F32, kind="ExternalOutput")

    with tile.TileContext(nc) as tc:
        with (
            tc.tile_pool(name="const", bufs=1) as cpool,
            tc.tile_pool(name="work", bufs=1) as wpool,
            tc.tile_pool(name="hdd", bufs=2) as hpool,
            tc.tile_pool(name="gscr", bufs=2) as spool,
            tc.tile_pool(name="pz", bufs=2, space="PSUM") as pz,
        ):
            # ---- PE warmup + ACT table preload (no input deps) ----
            warm = wpool.tile([128, 576], BF16)
            nc.gpsimd.memset(warm[:], 0.001)
            dum = wpool.tile([2, 4], F32)
            nc.scalar.activation(dum[:, 0:2], warm[0:2, 0:2], AF.Relu)
            pswarm = pz.tile([128, 2048], F32, tag="z")
            for r in range(WARMUP_MMS):
                nc.tensor.matmul(pswarm[0:64, 0:512], warm[:, 0:64],
                                 warm[:, 64:576], start=True, stop=True)

            # ---- inputs ----
            wb = cpool.tile([128, NB], BF16)
            nc.sync.dma_start(out=wb[:], in_=x_wb[:])
            pt = cpool.tile([128, 2, 2, 512], BF16)
            nc.gpsimd.dma_start(out=pt[:], in_=x_pt[:])
            wf = cpool.tile([128, NF], F32)
            nc.scalar.dma_start(out=wf[:], in_=x_wf[:])

            # ---- conv buffers + padding ----
            c1a = wpool.tile([128, 33, 33], BF16)   # imgs 0-3, ch at part 32k
            c1b = wpool.tile([64, 33, 33], BF16)    # imgs 4-5
            nc.gpsimd.memset(c1a[:, 32, :], 0.0)
            nc.gpsimd.memset(c1a[:, 0:32, 32], 0.0)
            nc.gpsimd.memset(c1b[:, 32, :], 0.0)
            nc.gpsimd.memset(c1b[:, 0:32, 32], 0.0)
            c2f = wpool.tile([128, 4, 17, 17], BF16)  # imgs0-3 @0:48, 4-5 @64:112
            nc.gpsimd.memset(c2f[:, :, 16, :], 0.0)
            nc.gpsimd.memset(c2f[:, :, 0:16, 16], 0.0)

            # ---- conv1: 27->32ch, 64x64 -> 32x32, diag-tiled 4 imgs ----
            c1ps = pz.tile([128, 2, 16, 32], F32, tag="z")
            for h in range(2):
                for k in range(4):
                    nc.tensor.matmul(
                        c1ps[32 * k:32 * k + 32, h, :, :],
                        wb[32 * k:32 * k + 27, W1C:W1C + 32],
                        pt[32 * k:32 * k + 27, h, 0, :],
                        start=True, stop=True, tile_position=(32 * k, 32 * k))
            nc.scalar.activation(
                c1a[:, 0:32, 0:32].rearrange("p (h y) x -> p h y x", h=2),
                c1ps[:], AF.Relu, bias=wf[:, BC1:BC1 + 1])
            c1psb = pz.tile([64, 2, 16, 32], F32, tag="z")
            for h in range(2):
                for k in range(2):
                    nc.tensor.matmul(
                        c1psb[32 * k:32 * k + 32, h, :, :],
                        wb[32 * k:32 * k + 27, W1C:W1C + 32],
                        pt[32 * k:32 * k + 27, h, 1, :],
                        start=True, stop=True)
            nc.vector.tensor_scalar(
                c1b[:, 0:32, 0:32].rearrange("p (h y) x -> p h y x", h=2),
                c1psb[:], wf[0:64, BC1:BC1 + 1], 0.0, op0=OP.add, op1=OP.max)

            # ---- conv2: 32->48ch, 32x32 -> 16x16, 4-way row-tiled ----
            c2ps = pz.tile([48, 4, 512], F32, tag="z")
            taps = [(dy, dx) for dy in range(3) for dx in range(3)]
            for t, (dy, dx) in enumerate(taps):
                for k in range(4):
                    nc.tensor.matmul(
                        c2ps[:, k, 0:256].rearrange("p (y x) -> p y x", y=16),
                        wb[32 * k:32 * k + 32, W2C + 48 * t:W2C + 48 * (t + 1)],
                        c1a[32 * k:32 * k + 32, dy:dy + 31:2, dx:dx + 31:2],
                        start=(t == 0), stop=(t == 8), skip_group_check=True,
                        tile_position=(32 * k, 0))
            nc.scalar.activation(
                c2f[0:48, :, 0:16, 0:16],
                c2ps[:, :, 0:256].rearrange("p k (y x) -> p k y x", y=16),
                AF.Relu, bias=wf[0:48, BC2:BC2 + 1])
            c2psb = pz.tile([128, 2, 512], F32, tag="z")
            for t, (dy, dx) in enumerate(taps):
                for k in range(2):
                    nc.tensor.matmul(
                        c2psb[64:112, k, 0:256].rearrange("p (y x) -> p y x", y=16),
                        wb[32 * k:32 * k + 32, W2C + 48 * t:W2C + 48 * (t + 1)],
                        c1b[32 * k:32 * k + 32, dy:dy + 31:2, dx:dx + 31:2],
                        start=(t == 0), stop=(t == 8), skip_group_check=True)
            nc.vector.tensor_scalar(
                c2f[64:112, 0:2, 0:16, 0:16],
                c2psb[64:112, :, 0:256].rearrange("p k (y x) -> p k y x", y=16),
                wf[64:112, BC2:BC2 + 1], 0.0, op0=OP.add, op1=OP.max)

            # ---- conv3: 48->64ch, 16x16 -> 8x8, 2 row-tile streams ----
            c3psa = pz.tile([64, 4, 64], F32, tag="z")
            c3psb = pz.tile([64, 2, 64], F32, tag="z")
            for t, (dy, dx) in enumerate(taps):
                nc.tensor.matmul(
                    c3psa[:].rearrange("p k (y x) -> p k y x", y=8),
                    wb[0:48, W3C + 64 * t:W3C + 64 * (t + 1)],
                    c2f[0:48, :, dy:dy + 15:2, dx:dx + 15:2],
                    start=(t == 0), stop=(t == 8), skip_group_check=True)
                nc.tensor.matmul(
                    c3psb[:].rearrange("p k (y x) -> p k y x", y=8),
                    wb[64:112, W3C + 64 * t:W3C + 64 * (t + 1)],
                    c2f[64:112, 0:2, dy:dy + 15:2, dx:dx + 15:2],
                    start=(t == 0), stop=(t == 8), skip_group_check=True)
            featc = wpool.tile([66, 6, M], BF16)
            nc.scalar.activation(featc[0:64, 0:4, :], c3psa[:], AF.Relu,
                                 bias=wf[0:64, BC3:BC3 + 1])
            nc.vector.tensor_scalar(featc[0:64, 4:6, :], c3psb[:],
                                    wf[0:64, BC3:BC3 + 1], 0.0,
                                    op0=OP.add, op1=OP.max)
            nc.vector.tensor_copy(featc[64:66, :, :],
                                  wb[64:66, CRD:CRD + 384]
                                  .rearrange("p (i m) -> p i m", m=M))

            # ---- u / v ----
            fc = featc[:].rearrange("p i m -> p (i m)")
            psu = pz.tile([128, 384], F32, tag="z")
            nc.tensor.matmul(psu[:], wb[0:66, W1A:W1A + 128], fc,
                             start=True, stop=True)
            psv = pz.tile([128, 384], F32, tag="z")
            nc.tensor.matmul(psv[:], wb[0:66, W1B:W1B + 128], fc,
                             start=True, stop=True)
            u_sb = wpool.tile([128, 384], F32)
            nc.scalar.activation(u_sb[:], psu[:], AF.Copy)
            v_bf = wpool.tile([128, 384], BF16)
            nc.vector.tensor_scalar(v_bf[:], psv[:], wf[:, BG1:BG1 + 1], None,
                                    op0=OP.add)

            # ---- cls head (runs whenever engines are free) ----
            fme = wpool.tile([65, S], F32)
            nc.gpsimd.memset(fme[64:65, :], 1.0)
            nc.vector.tensor_reduce(fme[0:64, :], featc[0:64, :, :],
                                    axis=mybir.AxisListType.X, op=OP.add)
            psl = pz.tile([S, NCls], F32, tag="z")
            nc.tensor.matmul(psl[:], fme[:], wf[0:65, WLE:WLE + 64],
                             start=True, stop=True)
            lsb = wpool.tile([S, NCls], F32)
            nc.vector.tensor_copy(lsb[:], psl[:])
            nc.sync.dma_start(out=out_cls[:], in_=lsb[:])

            # ---- relation units ----
            xf_cols = wpool.tile([128, 18], F32)
            for jl in range(3):
                for qh in range(2):
                    unit = jl * 2 + qh
                    act_qs = ACT_QS_BY_UNIT.get(unit, ())
                    hdd = hpool.tile([128, 32, 384], BF16, tag="hdd")
                    for ql in range(32):
                        q = qh * 32 + ql
                        ucol = u_sb[:, jl * M + q:jl * M + q + 1]
                        if ql in act_qs:
                            nc.scalar.activation(hdd[:, ql, :], v_bf[:],
                                                 AF.Relu, bias=ucol)
                        else:
                            nc.vector.tensor_scalar(hdd[:, ql, :], v_bf[:],
                                                    ucol, 0.0,
                                                    op0=OP.add, op1=OP.max)
                    for duo in range(3):
                        iA, iB = 2 * duo, 2 * duo + 1
                        zps = pz.tile([128, 2048], F32, tag="z")
                        for ch in range(4):
                            q0 = ch * 8
                            nc.tensor.matmul(
                                zps[0:CO, 512 * ch:512 * (ch + 1)],
                                wb[:, WG2:WG2 + 64],
                                hdd[:, q0:q0 + 8, iA * M:(iA + 1) * M],
                                start=True, stop=True)
                            nc.tensor.matmul(
                                zps[CO:2 * CO, 512 * ch:512 * (ch + 1)],
                                wb[:, WG2:WG2 + 64],
                                hdd[:, q0:q0 + 8, iB * M:(iB + 1) * M],
                                start=True, stop=True)
                        gscr = spool.tile([128, 2048], BF16, tag="gscr")
                        col = unit * 3 + duo
                        if (unit, duo) in DVE_Z:
                            nc.vector.tensor_scalar(
                                gscr[:], zps[:], wf[:, BG2:BG2 + 1], 0.0,
                                op0=OP.add, op1=OP.max,
                                accum_out=xf_cols[:, col:col + 1])
                        else:
                            nc.scalar.activation(
                                gscr[:], zps[:], AF.Relu,
                                bias=wf[:, BG2:BG2 + 1],
                                accum_out=xf_cols[:, col:col + 1])

            # ---- score head ----
            # xf_cols layout: col = (jl*2+qh)*3 + duo ; fold qh pairs
            xfq = wpool.tile([128, 9], F32)
            xv = xf_cols[:].rearrange("p (j h d) -> p j h d", j=3, h=2)
            nc.vector.tensor_tensor(
                xfq[:].rearrange("p (j d) -> p j d", j=3),
                xv[:, :, 0, :], xv[:, :, 1, :], op=OP.add)
            xfb = wpool.tile([128, 9], BF16)
            nc.vector.tensor_copy(xfb[:], xfq[:])
            psh1 = pz.tile([16, 18], F32, tag="z")
            nc.tensor.matmul(psh1[:, 0:9], wb[0:64, WF1:WF1 + 16],
                             xfb[0:64, :], start=True, stop=True)
            nc.tensor.matmul(psh1[:, 9:18], wb[64:128, WF1:WF1 + 16],
                             xfb[64:128, :], start=True, stop=True)
            h1s = wpool.tile([16, 18], BF16)
            nc.scalar.activation(h1s[:], psh1[:], AF.Relu,
                                 bias=wf[0:16, BF1:BF1 + 1])
            psh2 = pz.tile([18, 1], F32, tag="z")
            nc.tensor.matmul(psh2[:], h1s[:], wb[0:16, WF2:WF2 + 1],
                             start=True, stop=True)
            sc = wpool.tile([18, 1], F32)
            nc.vector.tensor_copy(sc[:], psh2[:])
            nc.sync.dma_start(out=out_scores[:], in_=sc[:])
    nc.compile()
    return nc


_NC_CACHE = None


def _get_nc():
    global _NC_CACHE
    if _NC_CACHE is None:
        _NC_CACHE = _build_nc()
    return _NC_CACHE


def _host_prep(inputs):
    f32 = np.float32
    bf16 = ml_dtypes.bfloat16
    ins = {k: np.asarray(v) for k, v in inputs.items()}
    x = np.concatenate([ins['support_x'], ins['query_x']], axis=1)
    lab = np.concatenate([ins['support_y'], ins['query_y']], axis=1)

    xpad = np.pad(x.astype(f32), ((0, 0), (0, 0), (0, 0), (0, 1), (0, 1)))
    win = np.lib.stride_tricks.sliding_window_view(xpad, (3, 3), axis=(3, 4))
    win = win[:, :, :, ::2, ::2]
    # [B, 27, S, 1024]
    patches = win.transpose(0, 2, 5, 6, 1, 3, 4).reshape(B, 27, S, 1024)
    patches = np.ascontiguousarray(patches, f32)

    w1 = np.ascontiguousarray(ins['k1'].reshape(32, 27).T, f32)
    w2 = np.ascontiguousarray(
        ins['k2'].transpose(1, 2, 3, 0).reshape(32, 9 * 48), f32)
    w3 = np.ascontiguousarray(
        ins['k3'].transpose(1, 2, 3, 0).reshape(48, 9 * 64), f32)
    Wg1 = ins['Wg1'].astype(f32)

    wb = np.zeros((128, NB), f32)
    for k in range(4):
        wb[32 * k:32 * k + 27, W1C:W1C + 32] = w1
        wb[32 * k:32 * k + 32, W2C:W2C + 432] = w2
    wb[0:48, W3C:W3C + 576] = w3
    wb[64:112, W3C:W3C + 576] = w3
    wb[0:66, W1A:W1A + 128] = Wg1[:C2]
    wb[0:66, W1B:W1B + 128] = Wg1[C2:]
    wb[0:128, WG2:WG2 + 64] = ins['Wg2'].astype(f32)
    wb[0:64, WF1:WF1 + 16] = ins['Wf1'].astype(f32)
    wb[64:128, WF1:WF1 + 16] = ins['Wf1'].astype(f32)
    wb[0:16, WF2:WF2 + 1] = ins['Wf2'].astype(f32)
    ii = np.arange(D, dtype=f32) / D
    coord = np.stack([np.broadcast_to(ii[:, None], (D, D)),
                      np.broadcast_to(ii[None, :], (D, D))]).reshape(2, M)
    wb[64:66, CRD:CRD + 384] = np.tile(coord, (1, S))
    wb = wb.astype(bf16)

    wfc = np.zeros((128, NF), f32)
    wfc[0:32, BC1] = ins['bc1'].astype(f32)
    wfc[0:48, BC2] = ins['bc2'].astype(f32)
    wfc[0:64, BC3] = ins['bc3'].astype(f32)
    wfc[64:128, BC3] = ins['bc3'].astype(f32)
    wfc[:, BG1] = ins['bg1'].astype(f32)
    wfc[0:64, BG2] = ins['bg2'].astype(f32)
    wfc[64:128, BG2] = ins['bg2'].astype(f32)
    wfc[0:16, BF1] = ins['bf1'].astype(f32)
    wfc[:, NBF2] = -float(ins['bf2'][0])
    wfc[0:64, WLE:WLE + 64] = ins['Wlog'].astype(f32) / M
    wfc[64, WLE:WLE + 64] = ins['blog'].astype(f32)

    onehots = np.zeros((B, S, NCls), f32)
    for b in range(B):
        onehots[b, np.arange(S), lab[b]] = 1.0

    in_maps = []
    for core in range(N_CORES):
        b, half = core // 2, core % 2
        perm = (0, 1, 2, 3, 4, 5) if half == 0 else (3, 4, 5, 0, 1, 2)
        p = patches[b][:, perm, :]          # [27, 6, 1024]
        pc = np.zeros((128, 2, 2, 512), f32)
        for k in range(4):
            pc[32 * k:32 * k + 27, :, 0, :] = \
                p[:, k, :].reshape(27, 2, 512)
        pc[0:27, :, 1, :] = p[:, 4, :].reshape(27, 2, 512)
        pc[32:59, :, 1, :] = p[:, 5, :].reshape(27, 2, 512)
        wfi = wfc.copy()
        wfi[0:S, OH:OH + 64] = onehots[b][list(perm)]
        in_maps.append(dict(wb=wb, wf=wfi, pt=pc.astype(bf16)))
    return in_maps, lab, ins['bf2'].astype(f32)


def _host_post(results, lab, bf2):
    P = np.zeros((B, S, S), np.float32)
    cls_terms = np.zeros((B, S), np.float32)
    for core in range(N_CORES):
        b, half = core // 2, core % 2
        perm = (0, 1, 2, 3, 4, 5) if half == 0 else (3, 4, 5, 0, 1, 2)
        raw = results[core]["scores"].reshape(18).astype(np.float64)
        sc = 1.0 / (1.0 + np.exp(-(raw + float(bf2[0]))))
        for r in range(18):
            ihalf, jl, duo = r // 9, (r % 9) // 3, r % 3
            P[b, perm[2 * duo + ihalf], perm[jl]] = np.float32(sc[r])
        if half == 0:
            lg = results[core]["logits"].reshape(S, NCls).astype(np.float64)
            mx = lg.max(axis=1, keepdims=True)
            lse = np.log(np.exp(lg - mx).sum(axis=1)) + mx[:, 0]
            cls_terms[b] = (lse - lg[np.arange(S), lab[b]]).astype(np.float32)
    cls_loss = np.float32(cls_terms.mean())
    y = (lab[:, :, None] == lab[:, None, :]).astype(np.float32)
    Pt = P.transpose(0, 2, 1)
    sym, anti = np.float32(0.5) * (P + Pt), np.float32(0.5) * (P - Pt)
    sym_n = np.sqrt((sym ** 2).sum(axis=(1, 2)))
    anti_n = np.sqrt((anti ** 2).sum(axis=(1, 2)))
    sym_loss = np.float32(((sym_n - anti_n) / (sym_n + anti_n)).mean())
    euc_loss = np.float32(((P - y) ** 2).mean())
    rn_loss = np.float32(euc_loss - np.float32(0.1) * sym_loss)
    return np.float32(cls_loss), np.float32(rn_loss), np.float32(sym_loss)


def run_spmd(inputs, trace=False, **kwargs):
    nc = _get_nc()
    in_maps, lab, bf2 = _host_prep(inputs)
    res = run_bass_kernel_spmd(nc, in_maps, list(range(N_CORES)),
                               trace=trace, **kwargs)
    return _host_post(res.results, lab, bf2), res


def kernel(**inputs):
    out, _ = run_spmd(inputs)
    return out
